# revision 1
# baseline (speedup 1.0000x reference)
"""GCN VGAE encoder (two GCNConv layers -> (mu, logstd)) on 8 Trainium2
NeuronCores via Bass/Tile.

Math: with deg = 1 + in_degree, dinv = deg^-1/2, and segment aggregation
S(u)[i] = sum_{e: dst e = i} u[src e]:
    u1 = (x @ W1) * dinv           h  = relu(dinv * (S(u1) + u1) + b1)
    u2 = h * dinv                  t  = dinv * (S(u2) + u2)
    mu = t @ W_mu + b_mu           ls = t @ W_ls + b_ls
(the linear transform commutes with segment_sum, so mu/ls share one S pass).

Distribution: nodes (and their incoming edges) sharded over 8 cores by dst
range; u1/u2 AllGathered so every core can gather arbitrary src rows.
Aggregation: edges bucketed by 128 dst rows; per 128-edge chunk a one-hot
selection matrix M[e,d] = (dstrel e == d) is built on DVE and the bucket's
PSUM accumulates M.T @ gathered_rows on the PE.
"""
import numpy as np

import concourse.bass as bass
import concourse.tile as tile
from concourse import mybir
from bass_rust import ScopedClock, SyncInfo

N_NODES = 100000
N_EDGES = 1600000
IN_CH, HID_CH, OUT_CH = 256, 64, 32
N_CORES = 8
NL = N_NODES // N_CORES          # 12500 nodes per core
NB = (NL + 127) // 128           # 98 dst buckets per core
LAST_ROWS = NL - (NB - 1) * 128  # rows in last bucket (84)
XT_COLS = NB * 128               # x^T padded to full tiles (12544)

# ---------------------------------------------------------------------------
# Workarounds for the walrus build in this container: it encodes at most ONE
# semaphore wait per instruction and rejects InstIncSwdgeSem.
# ---------------------------------------------------------------------------
_counter = [0]


def _patched_drain_and_barrier(self, tick_clock, wait_clock):
    drain_inst = self.nc.vector.drain()
    wait_clock.add_sem_waits(
        drain_inst.ins, ScopedClock({None: tick_clock.global_clock})
    )
    waits = list(drain_inst.ins.sync_info.on_wait)
    if len(waits) > 1:
        drain_inst.ins.sync_info = SyncInfo(on_wait=[waits[0]], on_update=[])
        for w in waits[1:]:
            extra = self.nc.vector.drain()
            extra.ins.sync_info = SyncInfo(on_wait=[w], on_update=[])
    self.nc.all_engine_barrier()
    assert self.sems is not None
    popped = self.nc._tile_sem_poison_stack.pop()
    assert popped is self._sem_poison
    self.nc.clear_and_free_semaphores(list(self.sems.allocated().values()))
    self.nc.all_engine_barrier()


tile.TileContext._drain_and_barrier = _patched_drain_and_barrier


def _steal_sem_clear(nc, first, last):
    cur = nc.cur_bb.bb
    inst = nc.gpsimd.sem_clear(range(first, last + 1))
    il = cur.instructions
    assert il and il[-1] is inst.ins
    cur.instructions = il[:-1]
    return inst.ins


def _fix_incswdge(nc):
    for bb in nc.main_func.blocks:
        il = bb.instructions
        if not any(type(i).__name__ == "InstIncSwdgeSem" for i in il):
            continue
        new_list = []
        for ins in il:
            if type(ins).__name__ != "InstIncSwdgeSem":
                new_list.append(ins)
                continue
            base = ins._sem_id_base
            values = list(ins._sem_values)
            names = list(ins._sem_names)
            si = ins.sync_info
            waits = list(si.on_wait) if si is not None else []
            for w in waits:
                _counter[0] += 1
                nop = mybir.InstNoOp(name=f"SWF-{_counter[0]}", ins=[], outs=[])
                nop.engine = ins.engine
                nop.sync_info = SyncInfo(on_wait=[w], on_update=[])
                new_list.append(nop)
            if ins._mode == "sub":
                nz = [k for k, v in enumerate(values) if v]
                if nz:
                    new_list.append(_steal_sem_clear(nc, base + min(nz), base + max(nz)))
            else:
                for k, v in enumerate(values):
                    for _ in range(v):
                        _counter[0] += 1
                        nop = mybir.InstNoOp(name=f"SWF-{_counter[0]}", ins=[], outs=[])
                        nop.engine = ins.engine
                        nop.sync_info = SyncInfo(
                            on_wait=[],
                            on_update=[
                                mybir.SyncUpdate(
                                    sync_type="semaphore", id=base + k,
                                    ant_name=names[k], update_mode="sem-inc",
                                    update_value=v * 0 + 1,
                                )
                            ],
                        )
                        new_list.append(nop)
        bb.instructions = new_list


def _split_multiwaits(nc):
    for bb in nc.main_func.blocks:
        il = bb.instructions
        if not any(
            i.sync_info is not None and len(i.sync_info.on_wait) > 1 for i in il
        ):
            continue
        new_list = []
        for ins in il:
            si = ins.sync_info
            waits = list(si.on_wait) if si is not None else []
            if len(waits) > 1:
                ups = list(si.on_update)
                for w in waits[:-1]:
                    _counter[0] += 1
                    nop = mybir.InstNoOp(name=f"WSP-{_counter[0]}", ins=[], outs=[])
                    nop.engine = ins.engine
                    nop.sync_info = SyncInfo(on_wait=[w], on_update=[])
                    new_list.append(nop)
                ins.sync_info = SyncInfo(on_wait=[waits[-1]], on_update=ups)
            new_list.append(ins)
        bb.instructions = new_list


# ---------------------------------------------------------------------------
# Device program
# ---------------------------------------------------------------------------
def _build_program(cpb):
    f32, i32 = mybir.dt.float32, mybir.dt.int32
    nc = bass.Bass("TRN2", target_bir_lowering=False, debug=False,
                   num_devices=N_CORES)
    C = NB * cpb  # chunks per core

    xT = nc.dram_tensor("xT", [NB, IN_CH, 128], f32, kind="ExternalInput")
    w1 = nc.dram_tensor("w1", [IN_CH, HID_CH], f32, kind="ExternalInput")
    wmuls = nc.dram_tensor("wmuls", [HID_CH, 2 * OUT_CH], f32, kind="ExternalInput")
    b1b = nc.dram_tensor("b1b", [128, HID_CH], f32, kind="ExternalInput")
    bmlb = nc.dram_tensor("bmlb", [128, 2 * OUT_CH], f32, kind="ExternalInput")
    dinvw = nc.dram_tensor("dinvw", [128, NB], f32, kind="ExternalInput")
    iota_in = nc.dram_tensor("iota_in", [128, 128], i32, kind="ExternalInput")
    ident_in = nc.dram_tensor("ident_in", [128, 128], f32, kind="ExternalInput")
    srcw = nc.dram_tensor("srcw", [128, C], i32, kind="ExternalInput")
    dstw = nc.dram_tensor("dstw", [128, C], i32, kind="ExternalInput")
    out = nc.dram_tensor("out", [NL, 2 * OUT_CH], f32, kind="ExternalOutput")

    u1b = nc.dram_tensor("u1b", [NL, HID_CH], f32)
    u1g = nc.dram_tensor("u1g", [N_NODES, HID_CH], f32)
    u2b = nc.dram_tensor("u2b", [NL, HID_CH], f32)
    u2g = nc.dram_tensor("u2g", [N_NODES, HID_CH], f32)

    with tile.TileContext(nc) as tc:
        with (
            tc.tile_pool(name="const", bufs=1) as cp,
            tc.tile_pool(name="slab", bufs=1) as sp,
            tc.tile_pool(name="xload", bufs=4) as xp,
            tc.tile_pool(name="work", bufs=8) as wp,
            tc.tile_pool(name="mt", bufs=8) as mp,
            tc.tile_pool(name="ep", bufs=4) as ep,
            tc.tile_pool(name="psv", bufs=2, space="PSUM") as ppv,
            tc.tile_pool(name="psg", bufs=2, space="PSUM") as ppg,
            tc.tile_pool(name="pst", bufs=2, space="PSUM") as ppt,
            tc.tile_pool(name="pso", bufs=2, space="PSUM") as ppo,
        ):
            # constants
            w1_sb = [cp.tile([128, HID_CH], f32, name=f"w1sb{k}")
                     for k in range(IN_CH // 128)]
            for k in range(IN_CH // 128):
                nc.sync.dma_start(out=w1_sb[k][:],
                                  in_=w1[k * 128:(k + 1) * 128, :])
            wml_sb = cp.tile([HID_CH, 2 * OUT_CH], f32)
            nc.sync.dma_start(out=wml_sb[:], in_=wmuls[:])
            b1_sb = cp.tile([128, HID_CH], f32)
            nc.sync.dma_start(out=b1_sb[:], in_=b1b[:])
            bml_sb = cp.tile([128, 2 * OUT_CH], f32)
            nc.sync.dma_start(out=bml_sb[:], in_=bmlb[:])
            dinv_sb = cp.tile([128, NB], f32)
            nc.sync.dma_start(out=dinv_sb[:], in_=dinvw[:])
            iota = cp.tile([128, 128], i32)
            nc.sync.dma_start(out=iota[:], in_=iota_in[:])
            ident = cp.tile([128, 128], f32)
            nc.sync.dma_start(out=ident[:], in_=ident_in[:])
            src_sb = sp.tile([128, C], i32)
            nc.sync.dma_start(out=src_sb[:], in_=srcw[:])
            dst_sb = sp.tile([128, C], i32)
            nc.sync.dma_start(out=dst_sb[:], in_=dstw[:])

            u1_slab = sp.tile([128, NB * HID_CH], f32)
            u2_slab = sp.tile([128, NB * HID_CH], f32)

            # phase 1: u1 = (x @ W1) * dinv, node-major tiles
            for m in range(NB):
                rows = 128 if m < NB - 1 else LAST_ROWS
                v_ps = ppv.tile([128, HID_CH], f32, tag="v")
                for k in range(IN_CH // 128):
                    xt_t = xp.tile([128, 128], f32, tag="xt")
                    nc.sync.dma_start(
                        out=xt_t[:], in_=xT[m, k * 128:(k + 1) * 128, :],
                    )
                    nc.tensor.matmul(
                        out=v_ps[:], lhsT=xt_t[:], rhs=w1_sb[k][:],
                        start=(k == 0), stop=(k == IN_CH // 128 - 1),
                    )
                u1_m = u1_slab[:, m * HID_CH:(m + 1) * HID_CH]
                nc.vector.tensor_scalar_mul(u1_m, v_ps[:], dinv_sb[:, m:m + 1])
                nc.sync.dma_start(
                    out=u1b[m * 128:m * 128 + rows, :], in_=u1_m[:rows, :]
                )

            nc.gpsimd.collective_compute(
                "AllGather", mybir.AluOpType.bypass,
                replica_groups=[list(range(N_CORES))],
                ins=[u1b[:].opt()], outs=[u1g[:].opt()],
            )

            def aggregation_pass(table, slab, out_slab_or_none):
                """One S() pass + fused epilogue per bucket. Returns nothing;
                writes h/u2 (pass 1) or final outputs (pass 2)."""
                for b in range(NB):
                    rows = 128 if b < NB - 1 else LAST_ROWS
                    g_ps = ppg.tile([128, HID_CH], f32, tag="g")
                    for j in range(cpb):
                        c = b * cpb + j
                        u_t = wp.tile([128, HID_CH], f32, tag="u")
                        nc.gpsimd.indirect_dma_start(
                            out=u_t[:], out_offset=None, in_=table[:],
                            in_offset=bass.IndirectOffsetOnAxis(
                                ap=src_sb[:, c:c + 1], axis=0),
                        )
                        m_t = mp.tile([128, 128], f32, tag="m")
                        nc.vector.tensor_tensor(
                            out=m_t[:],
                            in0=dst_sb[:, c:c + 1].to_broadcast([128, 128]),
                            in1=iota[:], op=mybir.AluOpType.is_equal,
                        )
                        nc.tensor.matmul(
                            out=g_ps[:], lhsT=m_t[:], rhs=u_t[:],
                            start=(j == 0), stop=(j == cpb - 1),
                        )
                    u_self = slab[:, b * HID_CH:(b + 1) * HID_CH]
                    s_t = ep.tile([128, HID_CH], f32, tag="s")
                    nc.vector.tensor_add(out=s_t[:], in0=g_ps[:], in1=u_self)
                    nc.vector.tensor_scalar_mul(s_t[:], s_t[:], dinv_sb[:, b:b + 1])
                    if out_slab_or_none is not None:
                        # pass 1 epilogue: h = relu(s + b1); u2 = h * dinv
                        nc.vector.tensor_add(out=s_t[:], in0=s_t[:], in1=b1_sb[:])
                        nc.scalar.activation(
                            s_t[:], s_t[:], mybir.ActivationFunctionType.Relu)
                        u2_m = out_slab_or_none[:, b * HID_CH:(b + 1) * HID_CH]
                        nc.vector.tensor_scalar_mul(
                            u2_m, s_t[:], dinv_sb[:, b:b + 1])
                        nc.sync.dma_start(
                            out=u2b[b * 128:b * 128 + rows, :], in_=u2_m[:rows, :])
                    else:
                        # pass 2 epilogue: out = t @ Wmuls + biases
                        tT_ps = ppt.tile([HID_CH, 128], f32, tag="tT")
                        nc.tensor.transpose(
                            out=tT_ps[:], in_=s_t[:], identity=ident[:])
                        tT_sb = ep.tile([HID_CH, 128], f32, tag="tTs")
                        nc.scalar.copy(out=tT_sb[:], in_=tT_ps[:])
                        o_ps = ppo.tile([128, 2 * OUT_CH], f32, tag="o")
                        nc.tensor.matmul(
                            out=o_ps[:], lhsT=tT_sb[:], rhs=wml_sb[:],
                            start=True, stop=True,
                        )
                        o_sb = ep.tile([128, 2 * OUT_CH], f32, tag="os")
                        nc.vector.tensor_add(out=o_sb[:], in0=o_ps[:], in1=bml_sb[:])
                        nc.sync.dma_start(
                            out=out[b * 128:b * 128 + rows, :], in_=o_sb[:rows, :])

            aggregation_pass(u1g, u1_slab, u2_slab)

            nc.gpsimd.collective_compute(
                "AllGather", mybir.AluOpType.bypass,
                replica_groups=[list(range(N_CORES))],
                ins=[u2b[:].opt()], outs=[u2g[:].opt()],
            )

            aggregation_pass(u2g, u2_slab, None)

    _fix_incswdge(nc)
    _split_multiwaits(nc)
    return nc


# ---------------------------------------------------------------------------
# Host-side sharding + launch
# ---------------------------------------------------------------------------
_cache = {}


def _prep(x, edge_index, W1, b1, W_mu, b_mu, W_ls, b_ls):
    x = np.asarray(x, np.float32)
    src = np.asarray(edge_index[0], np.int64).astype(np.int32)
    dst = np.asarray(edge_index[1], np.int64).astype(np.int32)

    deg = np.bincount(dst, minlength=N_NODES).astype(np.float32) + 1.0
    dinv = (1.0 / np.sqrt(deg)).astype(np.float32)

    core = dst // NL
    dst_rel = dst - core * NL
    bucket = dst_rel >> 7
    dst128 = dst_rel & 127

    # per (core,bucket) histogram -> uniform chunks-per-bucket
    gb = core.astype(np.int64) * NB + bucket
    counts = np.bincount(gb, minlength=N_CORES * NB)
    cpb = int((counts.max() + 127) // 128)
    C = NB * cpb

    order = np.argsort(gb, kind="stable")
    gb_s = gb[order]
    starts = np.zeros(N_CORES * NB + 1, np.int64)
    np.cumsum(counts, out=starts[1:])
    rank = np.arange(len(order), dtype=np.int64) - starts[gb_s]

    src_w = np.zeros((N_CORES, 128, C), np.int32)
    dst_w = np.full((N_CORES, 128, C), -1, np.int32)
    cc = gb_s // NB
    bb = gb_s % NB
    col = bb * cpb + rank // 128
    row = rank % 128
    src_w[cc, row, col] = src[order]
    dst_w[cc, row, col] = dst128[order]

    # x^T in tile-major layout: xT[c, m] = x_core[m*128:(m+1)*128].T (contig)
    xT = np.zeros((N_CORES, NB, IN_CH, 128), np.float32)
    xs = x.reshape(N_CORES, NL, IN_CH)
    xpad = np.zeros((N_CORES, XT_COLS, IN_CH), np.float32)
    xpad[:, :NL] = xs
    xT[:] = np.transpose(
        xpad.reshape(N_CORES, NB, 128, IN_CH), (0, 1, 3, 2))

    dinvw = np.ones((N_CORES, 128, NB), np.float32)
    dv = dinv.reshape(N_CORES, NL)
    for b in range(NB):
        rows = 128 if b < NB - 1 else LAST_ROWS
        dinvw[:, :rows, b] = dv[:, b * 128:b * 128 + rows]

    wmuls = np.concatenate([np.asarray(W_mu, np.float32),
                            np.asarray(W_ls, np.float32)], axis=1)
    bml = np.concatenate([np.asarray(b_mu, np.float32),
                          np.asarray(b_ls, np.float32)])[None, :]
    in_map_common = {
        "w1": np.asarray(W1, np.float32),
        "wmuls": wmuls,
        "b1b": np.broadcast_to(np.asarray(b1, np.float32)[None, :],
                               (128, HID_CH)).copy(),
        "bmlb": np.broadcast_to(bml, (128, 2 * OUT_CH)).copy(),
        "iota_in": np.broadcast_to(np.arange(128, dtype=np.int32)[None, :],
                                   (128, 128)).copy(),
        "ident_in": np.eye(128, dtype=np.float32),
    }
    in_maps = []
    for c in range(N_CORES):
        m = dict(in_map_common)
        m["xT"] = xT[c]
        m["srcw"] = src_w[c]
        m["dstw"] = dst_w[c]
        m["dinvw"] = dinvw[c]
        in_maps.append(m)
    return cpb, in_maps


def kernel(x, edge_index, W1, b1, W_mu, b_mu, W_ls, b_ls):
    from concourse.bass_utils import run_bass_kernel_spmd

    cpb, in_maps = _prep(x, edge_index, W1, b1, W_mu, b_mu, W_ls, b_ls)
    if cpb not in _cache:
        _cache[cpb] = _build_program(cpb)
    nc = _cache[cpb]
    res = run_bass_kernel_spmd(nc, in_maps, list(range(N_CORES)))
    full = np.concatenate([res.results[c]["out"] for c in range(N_CORES)], axis=0)
    return full[:, :OUT_CH].copy(), full[:, OUT_CH:].copy()



# revision 5
# speedup vs baseline: 1.9684x; 1.9684x over previous
"""GCN VGAE encoder (two GCNConv layers -> (mu, logstd)) on 8 Trainium2
NeuronCores via Bass/Tile — v2.

Math: with deg = 1 + in_degree, dinv = deg^-1/2, S(u)[i] = sum_{dst e = i} u[src e]:
    u1 = (x @ W1) * dinv           h  = relu(dinv * (S(u1) + u1) + b1)
    u2 = h * dinv                  t  = dinv * (S(u2) + u2)
    mu = t @ W_mu + b_mu           ls = t @ W_ls + b_ls

v2 changes vs v1:
  - bf16 tables/operands everywhere (f32 accumulation in PSUM/SBUF).
  - edges per (bucket, src-segment) with variable chunk counts; src rows
    remapped into per-segment AllGather outputs (G=2 segments) so each
    AllGather overlaps the producer loop's tail and the consumer loop
    starts after only its segment arrived.
  - AllGather outputs use Shared address space.
  - one-hot selection matrices built in one DVE op per (bucket, segment).
  - epilogues fused into the last segment sweep.
"""
import numpy as np

import concourse.bass as bass
import concourse.tile as tile
from concourse import mybir
from bass_rust import ScopedClock, SyncInfo

N_NODES = 100000
N_EDGES = 1600000
IN_CH, HID_CH, OUT_CH = 256, 64, 32
N_CORES = 8
NL = N_NODES // N_CORES          # 12500 nodes per core
NB = (NL + 127) // 128           # 98 dst buckets per core
LAST_ROWS = NL - (NB - 1) * 128  # rows in last bucket (84)
G = 2                            # src segments (AllGather pipeline depth)
SEGL = NL // G                   # 6250 local rows per segment
SEG_ROWS = N_CORES * SEGL        # rows per gathered segment table

# ---------------------------------------------------------------------------
# Workarounds for the walrus build in this container: it encodes at most ONE
# semaphore wait per instruction and rejects InstIncSwdgeSem.
# ---------------------------------------------------------------------------
_counter = [0]


def _patched_drain_and_barrier(self, tick_clock, wait_clock):
    drain_inst = self.nc.vector.drain()
    wait_clock.add_sem_waits(
        drain_inst.ins, ScopedClock({None: tick_clock.global_clock})
    )
    waits = list(drain_inst.ins.sync_info.on_wait)
    if len(waits) > 1:
        drain_inst.ins.sync_info = SyncInfo(on_wait=[waits[0]], on_update=[])
        for w in waits[1:]:
            extra = self.nc.vector.drain()
            extra.ins.sync_info = SyncInfo(on_wait=[w], on_update=[])
    self.nc.all_engine_barrier()
    assert self.sems is not None
    popped = self.nc._tile_sem_poison_stack.pop()
    assert popped is self._sem_poison
    self.nc.clear_and_free_semaphores(list(self.sems.allocated().values()))
    self.nc.all_engine_barrier()


tile.TileContext._drain_and_barrier = _patched_drain_and_barrier


def _steal_sem_clear(nc, first, last):
    cur = nc.cur_bb.bb
    inst = nc.gpsimd.sem_clear(range(first, last + 1))
    il = cur.instructions
    assert il and il[-1] is inst.ins
    cur.instructions = il[:-1]
    return inst.ins


def _fix_incswdge(nc):
    for bb in nc.main_func.blocks:
        il = bb.instructions
        if not any(type(i).__name__ == "InstIncSwdgeSem" for i in il):
            continue
        new_list = []
        for ins in il:
            if type(ins).__name__ != "InstIncSwdgeSem":
                new_list.append(ins)
                continue
            base = ins._sem_id_base
            values = list(ins._sem_values)
            names = list(ins._sem_names)
            si = ins.sync_info
            waits = list(si.on_wait) if si is not None else []
            for w in waits:
                _counter[0] += 1
                nop = mybir.InstNoOp(name=f"SWF-{_counter[0]}", ins=[], outs=[])
                nop.engine = ins.engine
                nop.sync_info = SyncInfo(on_wait=[w], on_update=[])
                new_list.append(nop)
            if ins._mode == "sub":
                nz = [k for k, v in enumerate(values) if v]
                if nz:
                    new_list.append(_steal_sem_clear(nc, base + min(nz), base + max(nz)))
            else:
                for k, v in enumerate(values):
                    for _ in range(v):
                        _counter[0] += 1
                        nop = mybir.InstNoOp(name=f"SWF-{_counter[0]}", ins=[], outs=[])
                        nop.engine = ins.engine
                        nop.sync_info = SyncInfo(
                            on_wait=[],
                            on_update=[
                                mybir.SyncUpdate(
                                    sync_type="semaphore", id=base + k,
                                    ant_name=names[k], update_mode="sem-inc",
                                    update_value=v * 0 + 1,
                                )
                            ],
                        )
                        new_list.append(nop)
        bb.instructions = new_list


def _split_multiwaits(nc):
    for bb in nc.main_func.blocks:
        il = bb.instructions
        if not any(
            i.sync_info is not None and len(i.sync_info.on_wait) > 1 for i in il
        ):
            continue
        new_list = []
        for ins in il:
            si = ins.sync_info
            waits = list(si.on_wait) if si is not None else []
            if len(waits) > 1:
                ups = list(si.on_update)
                for w in waits[:-1]:
                    _counter[0] += 1
                    nop = mybir.InstNoOp(name=f"WSP-{_counter[0]}", ins=[], outs=[])
                    nop.engine = ins.engine
                    nop.sync_info = SyncInfo(on_wait=[w], on_update=[])
                    new_list.append(nop)
                ins.sync_info = SyncInfo(on_wait=[waits[-1]], on_update=ups)
            new_list.append(ins)
        bb.instructions = new_list


# ---------------------------------------------------------------------------
# Device program
# ---------------------------------------------------------------------------
def _build_program(kmat):
    """kmat: [NB][G] chunk counts (shared across cores)."""
    f32, bf16, i32 = mybir.dt.float32, mybir.dt.bfloat16, mybir.dt.int32
    nc = bass.Bass("TRN2", target_bir_lowering=False, debug=False,
                   num_devices=N_CORES)
    C = int(sum(kmat[b][s] for b in range(NB) for s in range(G)))
    # column offset of each (s, b) chunk group in srcw/dstw
    col0 = {}
    c = 0
    for s in range(G):
        for b in range(NB):
            col0[(s, b)] = c
            c += kmat[b][s]
    assert c == C

    xT = nc.dram_tensor("xT", [NB, IN_CH, 128], bf16, kind="ExternalInput")
    w1 = nc.dram_tensor("w1", [IN_CH, HID_CH], bf16, kind="ExternalInput")
    wmuls = nc.dram_tensor("wmuls", [HID_CH, 2 * OUT_CH], bf16, kind="ExternalInput")
    b1b = nc.dram_tensor("b1b", [128, HID_CH], f32, kind="ExternalInput")
    bmlb = nc.dram_tensor("bmlb", [128, 2 * OUT_CH], f32, kind="ExternalInput")
    dinvw = nc.dram_tensor("dinvw", [128, NB], f32, kind="ExternalInput")
    iota_in = nc.dram_tensor("iota_in", [128, 128], i32, kind="ExternalInput")
    ident_in = nc.dram_tensor("ident_in", [128, 128], bf16, kind="ExternalInput")
    srcw = nc.dram_tensor("srcw", [128, C], i32, kind="ExternalInput")
    dstw = nc.dram_tensor("dstw", [128, C], i32, kind="ExternalInput")
    out = nc.dram_tensor("out", [NL, 2 * OUT_CH], f32, kind="ExternalOutput")

    ub = [[nc.dram_tensor(f"u{p}b{s}", [SEGL, HID_CH], bf16)
           for s in range(G)] for p in (1, 2)]
    ug = [[nc.dram_tensor(f"u{p}g{s}", [SEG_ROWS, HID_CH], bf16,
                          addr_space="Shared")
           for s in range(G)] for p in (1, 2)]

    with tile.TileContext(nc) as tc:
        with (
            tc.tile_pool(name="const", bufs=1) as cp,
            tc.tile_pool(name="slab", bufs=1) as sp,
            tc.tile_pool(name="xload", bufs=4) as xp,
            tc.tile_pool(name="work", bufs=8) as wp,
            tc.tile_pool(name="mt", bufs=2) as mp,
            tc.tile_pool(name="ep", bufs=4) as ep,
            tc.tile_pool(name="psv", bufs=2, space="PSUM") as ppv,
            tc.tile_pool(name="psg", bufs=2, space="PSUM") as ppg,
            tc.tile_pool(name="pst", bufs=2, space="PSUM") as ppt,
            tc.tile_pool(name="pso", bufs=2, space="PSUM") as ppo,
        ):
            # constants
            w1_sb = [cp.tile([128, HID_CH], bf16, name=f"w1sb{k}")
                     for k in range(IN_CH // 128)]
            for k in range(IN_CH // 128):
                nc.sync.dma_start(out=w1_sb[k][:],
                                  in_=w1[k * 128:(k + 1) * 128, :])
            wml_sb = cp.tile([HID_CH, 2 * OUT_CH], bf16)
            nc.sync.dma_start(out=wml_sb[:], in_=wmuls[:])
            b1_sb = cp.tile([128, HID_CH], f32)
            nc.sync.dma_start(out=b1_sb[:], in_=b1b[:])
            bml_sb = cp.tile([128, 2 * OUT_CH], f32)
            nc.sync.dma_start(out=bml_sb[:], in_=bmlb[:])
            dinv_sb = cp.tile([128, NB], f32)
            nc.sync.dma_start(out=dinv_sb[:], in_=dinvw[:])
            iota = cp.tile([128, 128], i32)
            nc.sync.dma_start(out=iota[:], in_=iota_in[:])
            ident = cp.tile([128, 128], bf16)
            nc.sync.dma_start(out=ident[:], in_=ident_in[:])
            src_sb = sp.tile([128, C], i32)
            nc.sync.dma_start(out=src_sb[:], in_=srcw[:])
            dst_sb = sp.tile([128, C], i32)
            nc.sync.dma_start(out=dst_sb[:], in_=dstw[:])

            u1_slab = sp.tile([128, NB * HID_CH], bf16)
            u2_slab = sp.tile([128, NB * HID_CH], bf16)
            acc_slab = sp.tile([128, NB * HID_CH], f32)

            def store_rows(slab_col, b, rows, dest_pair):
                """DMA bucket b's node-major rows into segment tensors."""
                lo, hi = b * 128, b * 128 + rows
                for s in range(G):
                    s0, s1 = s * SEGL, (s + 1) * SEGL
                    a, z = max(lo, s0), min(hi, s1)
                    if a < z:
                        nc.sync.dma_start(
                            out=dest_pair[s][a - s0:z - s0, :],
                            in_=slab_col[a - lo:z - lo, :],
                        )

            def emit_ag(p, s):
                nc.gpsimd.collective_compute(
                    "AllGather", mybir.AluOpType.bypass,
                    replica_groups=[list(range(N_CORES))],
                    ins=[ub[p][s][:].opt()], outs=[ug[p][s][:].opt()],
                )

            # segment s's last producing bucket
            ag_after = [min(NB - 1, ((s + 1) * SEGL + 127) // 128 - 1)
                        for s in range(G)]

            # ---- phase 1: u1 = (x @ W1) * dinv ----
            for m in range(NB):
                rows = 128 if m < NB - 1 else LAST_ROWS
                v_ps = ppv.tile([128, HID_CH], f32, tag="v")
                for k in range(IN_CH // 128):
                    xt_t = xp.tile([128, 128], bf16, tag="xt")
                    nc.sync.dma_start(
                        out=xt_t[:], in_=xT[m, k * 128:(k + 1) * 128, :],
                    )
                    nc.tensor.matmul(
                        out=v_ps[:], lhsT=xt_t[:], rhs=w1_sb[k][:],
                        start=(k == 0), stop=(k == IN_CH // 128 - 1),
                    )
                u1_m = u1_slab[:, m * HID_CH:(m + 1) * HID_CH]
                nc.vector.tensor_scalar_mul(u1_m, v_ps[:], dinv_sb[:, m:m + 1])
                store_rows(u1_m, m, rows, ub[0])
                for s in range(G):
                    if m == ag_after[s]:
                        emit_ag(0, s)

            # ---- aggregation passes ----
            def aggregation_pass(pidx, self_slab, is_first_pass):
                for s in range(G):
                    table = ug[pidx][s]
                    for b in range(NB):
                        K = kmat[b][s]
                        if K == 0:
                            continue
                        c0 = col0[(s, b)]
                        m_slab = mp.tile([128, K * 128], bf16, tag="m")
                        nc.vector.tensor_tensor(
                            out=m_slab[:].rearrange("p (k d) -> p k d", k=K),
                            in0=dst_sb[:, c0:c0 + K].to_broadcast([128, K, 128]),
                            in1=iota[:].unsqueeze(1).broadcast_to([128, K, 128]),
                            op=mybir.AluOpType.is_equal,
                        )
                        g_ps = ppg.tile([128, HID_CH], f32, tag="g")
                        for j in range(K):
                            u_t = wp.tile([128, HID_CH], bf16, tag="u")
                            nc.gpsimd.indirect_dma_start(
                                out=u_t[:], out_offset=None, in_=table[:],
                                in_offset=bass.IndirectOffsetOnAxis(
                                    ap=src_sb[:, c0 + j:c0 + j + 1], axis=0),
                            )
                            nc.tensor.matmul(
                                out=g_ps[:],
                                lhsT=m_slab[:, j * 128:(j + 1) * 128],
                                rhs=u_t[:],
                                start=(j == 0), stop=(j == K - 1),
                            )
                        acc_b = acc_slab[:, b * HID_CH:(b + 1) * HID_CH]
                        if s == 0:
                            nc.vector.tensor_copy(out=acc_b, in_=g_ps[:])
                        else:
                            nc.vector.tensor_add(out=acc_b, in0=acc_b, in1=g_ps[:])
                        if s == G - 1:
                            epilogue(b, is_first_pass, self_slab)

            def epilogue(b, is_first_pass, self_slab):
                rows = 128 if b < NB - 1 else LAST_ROWS
                acc_b = acc_slab[:, b * HID_CH:(b + 1) * HID_CH]
                u_self = self_slab[:, b * HID_CH:(b + 1) * HID_CH]
                s_t = ep.tile([128, HID_CH], f32, tag="s")
                nc.vector.tensor_add(out=s_t[:], in0=acc_b, in1=u_self)
                nc.vector.tensor_scalar_mul(s_t[:], s_t[:], dinv_sb[:, b:b + 1])
                if is_first_pass:
                    nc.vector.tensor_add(out=s_t[:], in0=s_t[:], in1=b1_sb[:])
                    nc.scalar.activation(
                        s_t[:], s_t[:], mybir.ActivationFunctionType.Relu)
                    u2_m = u2_slab[:, b * HID_CH:(b + 1) * HID_CH]
                    nc.vector.tensor_scalar_mul(u2_m, s_t[:], dinv_sb[:, b:b + 1])
                    store_rows(u2_m, b, rows, ub[1])
                    for s in range(G):
                        if b == ag_after[s]:
                            emit_ag(1, s)
                else:
                    t_bf = ep.tile([128, HID_CH], bf16, tag="tb")
                    nc.vector.tensor_copy(out=t_bf[:], in_=s_t[:])
                    tT_ps = ppt.tile([HID_CH, 128], bf16, tag="tT")
                    nc.tensor.transpose(out=tT_ps[:], in_=t_bf[:], identity=ident[:])
                    tT_sb = ep.tile([HID_CH, 128], bf16, tag="tTs")
                    nc.scalar.copy(out=tT_sb[:], in_=tT_ps[:])
                    o_ps = ppo.tile([128, 2 * OUT_CH], f32, tag="o")
                    nc.tensor.matmul(
                        out=o_ps[:], lhsT=tT_sb[:], rhs=wml_sb[:],
                        start=True, stop=True,
                    )
                    o_sb = ep.tile([128, 2 * OUT_CH], f32, tag="os")
                    nc.vector.tensor_add(out=o_sb[:], in0=o_ps[:], in1=bml_sb[:])
                    nc.sync.dma_start(
                        out=out[b * 128:b * 128 + rows, :], in_=o_sb[:rows, :])

            aggregation_pass(0, u1_slab, True)
            aggregation_pass(1, u2_slab, False)

    _fix_incswdge(nc)
    _split_multiwaits(nc)
    return nc


# ---------------------------------------------------------------------------
# Host-side sharding + launch
# ---------------------------------------------------------------------------
_cache = {}


def _prep(x, edge_index, W1, b1, W_mu, b_mu, W_ls, b_ls):
    import ml_dtypes

    bf16 = ml_dtypes.bfloat16
    x = np.asarray(x, np.float32)
    src = np.asarray(edge_index[0]).astype(np.int64)
    dst = np.asarray(edge_index[1]).astype(np.int64)

    deg = np.bincount(dst, minlength=N_NODES).astype(np.float32) + 1.0
    dinv = (1.0 / np.sqrt(deg)).astype(np.float32)

    core = dst // NL
    dst_rel = dst - core * NL
    bucket = dst_rel >> 7
    dst128 = (dst_rel & 127).astype(np.int32)

    sl = src % NL
    seg = sl // SEGL
    srow = ((src // NL) * SEGL + (sl - seg * SEGL)).astype(np.int32)

    # group edges by (core, seg, bucket)
    gkey = (core * G + seg) * NB + bucket
    counts = np.bincount(gkey, minlength=N_CORES * G * NB).reshape(
        N_CORES, G, NB)
    kmat = np.maximum((counts.max(axis=0) + 127) // 128, 1).T  # [NB][G]
    kmat = kmat.astype(np.int64)

    order = np.argsort(gkey, kind="stable")
    gk_s = gkey[order]
    starts = np.zeros(N_CORES * G * NB + 1, np.int64)
    np.cumsum(counts.reshape(-1), out=starts[1:])
    rank = np.arange(len(order), dtype=np.int64) - starts[gk_s]

    # column offsets per (seg, bucket), shared by all cores
    kcum = np.zeros(G * NB + 1, np.int64)
    ks = np.stack([kmat[:, s] for s in range(G)]).reshape(-1)  # s-major
    np.cumsum(ks, out=kcum[1:])
    C = int(kcum[-1])

    cc = gk_s // (G * NB)
    sb = gk_s % (G * NB)  # seg*NB + bucket
    col = kcum[sb] + rank // 128
    row = rank % 128

    src_w = np.zeros((N_CORES, 128, C), np.int32)
    dst_w = np.full((N_CORES, 128, C), -1, np.int32)
    src_w[cc, row, col] = srow[order]
    dst_w[cc, row, col] = dst128[order]

    # x^T in tile-major layout (bf16)
    xs = x.reshape(N_CORES, NL, IN_CH)
    xpad = np.zeros((N_CORES, NB * 128, IN_CH), np.float32)
    xpad[:, :NL] = xs
    xT = np.ascontiguousarray(
        np.transpose(xpad.reshape(N_CORES, NB, 128, IN_CH), (0, 1, 3, 2))
    ).astype(bf16)

    dinvw = np.ones((N_CORES, 128, NB), np.float32)
    dv = dinv.reshape(N_CORES, NL)
    for b in range(NB):
        rows = 128 if b < NB - 1 else LAST_ROWS
        dinvw[:, :rows, b] = dv[:, b * 128:b * 128 + rows]

    wmuls = np.concatenate([np.asarray(W_mu, np.float32),
                            np.asarray(W_ls, np.float32)], axis=1).astype(bf16)
    bml = np.concatenate([np.asarray(b_mu, np.float32),
                          np.asarray(b_ls, np.float32)])[None, :]
    in_map_common = {
        "w1": np.asarray(W1, np.float32).astype(bf16),
        "wmuls": wmuls,
        "b1b": np.broadcast_to(np.asarray(b1, np.float32)[None, :],
                               (128, HID_CH)).copy(),
        "bmlb": np.broadcast_to(bml, (128, 2 * OUT_CH)).astype(np.float32).copy(),
        "iota_in": np.broadcast_to(np.arange(128, dtype=np.int32)[None, :],
                                   (128, 128)).copy(),
        "ident_in": np.eye(128, dtype=np.float32).astype(bf16),
    }
    in_maps = []
    for c in range(N_CORES):
        m = dict(in_map_common)
        m["xT"] = xT[c]
        m["srcw"] = src_w[c]
        m["dstw"] = dst_w[c]
        m["dinvw"] = dinvw[c]
        in_maps.append(m)
    return kmat, in_maps


def kernel(x, edge_index, W1, b1, W_mu, b_mu, W_ls, b_ls):
    from concourse.bass_utils import run_bass_kernel_spmd

    kmat, in_maps = _prep(x, edge_index, W1, b1, W_mu, b_mu, W_ls, b_ls)
    key = kmat.tobytes()
    if key not in _cache:
        _cache[key] = _build_program(kmat)
    nc = _cache[key]
    res = run_bass_kernel_spmd(nc, in_maps, list(range(N_CORES)))
    full = np.concatenate([res.results[c]["out"] for c in range(N_CORES)], axis=0)
    return full[:, :OUT_CH].copy(), full[:, OUT_CH:].copy()


# revision 8
# speedup vs baseline: 6.3369x; 3.2194x over previous
"""GCN VGAE encoder (two GCNConv layers -> (mu, logstd)) on 8 Trainium2
NeuronCores via Bass/Tile — v2.

Math: with deg = 1 + in_degree, dinv = deg^-1/2, S(u)[i] = sum_{dst e = i} u[src e]:
    u1 = (x @ W1) * dinv           h  = relu(dinv * (S(u1) + u1) + b1)
    u2 = h * dinv                  t  = dinv * (S(u2) + u2)
    mu = t @ W_mu + b_mu           ls = t @ W_ls + b_ls

v2 changes vs v1:
  - bf16 tables/operands everywhere (f32 accumulation in PSUM/SBUF).
  - edges per (bucket, src-segment) with variable chunk counts; src rows
    remapped into per-segment AllGather outputs (G=2 segments) so each
    AllGather overlaps the producer loop's tail and the consumer loop
    starts after only its segment arrived.
  - AllGather outputs use Shared address space.
  - one-hot selection matrices built in one DVE op per (bucket, segment).
  - epilogues fused into the last segment sweep.
"""
import io
import tarfile

import numpy as np

import concourse.bass as bass
import concourse.bass2jax as _b2j
import concourse.tile as tile
from concourse import mybir
from concourse import neff as _neffmod
from bass_rust import ScopedClock, SyncInfo

N_NODES = 100000
N_EDGES = 1600000
IN_CH, HID_CH, OUT_CH = 256, 64, 32
N_CORES = 8
NL = N_NODES // N_CORES          # 12500 nodes per core
NB = (NL + 127) // 128           # 98 dst buckets per core
LAST_ROWS = NL - (NB - 1) * 128  # rows in last bucket (84)
G = 2                            # src segments (AllGather pipeline depth)
SEGL = NL // G                   # 6250 local rows per segment
SEG_ROWS = N_CORES * SEGL        # rows per gathered segment table

# ---------------------------------------------------------------------------
# Workarounds for the walrus build in this container: it encodes at most ONE
# semaphore wait per instruction and rejects InstIncSwdgeSem.
# ---------------------------------------------------------------------------
_counter = [0]


def _patched_drain_and_barrier(self, tick_clock, wait_clock):
    drain_inst = self.nc.vector.drain()
    wait_clock.add_sem_waits(
        drain_inst.ins, ScopedClock({None: tick_clock.global_clock})
    )
    waits = list(drain_inst.ins.sync_info.on_wait)
    if len(waits) > 1:
        drain_inst.ins.sync_info = SyncInfo(on_wait=[waits[0]], on_update=[])
        for w in waits[1:]:
            extra = self.nc.vector.drain()
            extra.ins.sync_info = SyncInfo(on_wait=[w], on_update=[])
    self.nc.all_engine_barrier()
    assert self.sems is not None
    popped = self.nc._tile_sem_poison_stack.pop()
    assert popped is self._sem_poison
    self.nc.clear_and_free_semaphores(list(self.sems.allocated().values()))
    self.nc.all_engine_barrier()


tile.TileContext._drain_and_barrier = _patched_drain_and_barrier


def _steal_sem_clear(nc, first, last):
    cur = nc.cur_bb.bb
    inst = nc.gpsimd.sem_clear(range(first, last + 1))
    il = cur.instructions
    assert il and il[-1] is inst.ins
    cur.instructions = il[:-1]
    return inst.ins


def _fix_incswdge(nc):
    for bb in nc.main_func.blocks:
        il = bb.instructions
        if not any(type(i).__name__ == "InstIncSwdgeSem" for i in il):
            continue
        new_list = []
        for ins in il:
            if type(ins).__name__ != "InstIncSwdgeSem":
                new_list.append(ins)
                continue
            base = ins._sem_id_base
            values = list(ins._sem_values)
            names = list(ins._sem_names)
            si = ins.sync_info
            waits = list(si.on_wait) if si is not None else []
            for w in waits:
                _counter[0] += 1
                nop = mybir.InstNoOp(name=f"SWF-{_counter[0]}", ins=[], outs=[])
                nop.engine = ins.engine
                nop.sync_info = SyncInfo(on_wait=[w], on_update=[])
                new_list.append(nop)
            if ins._mode == "sub":
                nz = [k for k, v in enumerate(values) if v]
                if nz:
                    new_list.append(_steal_sem_clear(nc, base + min(nz), base + max(nz)))
            else:
                for k, v in enumerate(values):
                    for _ in range(v):
                        _counter[0] += 1
                        nop = mybir.InstNoOp(name=f"SWF-{_counter[0]}", ins=[], outs=[])
                        nop.engine = ins.engine
                        nop.sync_info = SyncInfo(
                            on_wait=[],
                            on_update=[
                                mybir.SyncUpdate(
                                    sync_type="semaphore", id=base + k,
                                    ant_name=names[k], update_mode="sem-inc",
                                    update_value=v * 0 + 1,
                                )
                            ],
                        )
                        new_list.append(nop)
        bb.instructions = new_list


def _split_multiwaits(nc):
    for bb in nc.main_func.blocks:
        il = bb.instructions
        if not any(
            i.sync_info is not None and len(i.sync_info.on_wait) > 1 for i in il
        ):
            continue
        new_list = []
        for ins in il:
            si = ins.sync_info
            waits = list(si.on_wait) if si is not None else []
            if len(waits) > 1:
                ups = list(si.on_update)
                for w in waits[:-1]:
                    _counter[0] += 1
                    nop = mybir.InstNoOp(name=f"WSP-{_counter[0]}", ins=[], outs=[])
                    nop.engine = ins.engine
                    nop.sync_info = SyncInfo(on_wait=[w], on_update=[])
                    new_list.append(nop)
                ins.sync_info = SyncInfo(on_wait=[waits[-1]], on_update=ups)
            new_list.append(ins)
        bb.instructions = new_list


# ---------------------------------------------------------------------------
# NEFF post-patch: widen each per-group indirect gather from 128 indices x
# (K*128B) blocks to (K*128) indices x 128B blocks.  The dst fields keep the
# walrus-emitted shape (the NEFF loader validates them); the gathered stream
# then fills each partition's K blocks consecutively, which the host-side
# index remap in _prep accounts for.
# ---------------------------------------------------------------------------
_widen_plan = []  # K per indirect-DMA instruction, in program order


def _patch_neff(neff_bytes):
    if not _widen_plan:
        return neff_bytes
    hdr, tar_data = neff_bytes[:1024], bytearray(neff_bytes[1024:])
    tf = tarfile.open(fileobj=io.BytesIO(bytes(tar_data)))
    member = None
    for nm in tf.getnames():
        if nm.endswith("sg00/Pool0.bin"):
            member = tf.getmember(nm)
            break
    if member is None:
        return neff_bytes
    off, size = member.offset_data, member.size
    probe_nc = bass.Bass("TRN2", target_bir_lowering=False, debug=False,
                         num_devices=1)
    ffi = probe_nc.isa.ffi
    Op = probe_nc.isa.Opcode
    d2d = int(Op.NEURON_ISA_TPB_OPCODE_PSEUDO_DMA_DIRECT2D.value)
    indirect_at = []
    for i in range(size // 64):
        base = off + i * 64
        blob = bytes(tar_data[base:base + 64])
        if blob[0] != d2d:
            continue
        p = ffi.cast("NEURON_ISA_TPB_PSEUDO_DMA_DIRECT2D_STRUCT *",
                     ffi.from_buffer(blob))
        if int(p.dge_op) == 1:
            indirect_at.append(base)
    if len(indirect_at) != len(_widen_plan):
        return neff_bytes  # not our program
    for base, K in zip(indirect_at, _widen_plan):
        buf = bytearray(bytes(tar_data[base:base + 64]))
        q = ffi.cast("NEURON_ISA_TPB_PSEUDO_DMA_DIRECT2D_STRUCT *",
                     ffi.from_buffer(buf))
        assert int(q.src_num_elem[0]) == 128
        assert int(q.src_elem_size) == K * 128, (int(q.src_elem_size), K)
        q.src_num_elem[0] = 128 * K
        q.src_elem_size = 128
        tar_data[base:base + 64] = bytes(buf)
    new_tar = bytes(tar_data)
    new_hdr = _neffmod.make_deterministic_neff_header(hdr, new_tar)
    return new_hdr + new_tar


_orig_rename = _b2j.rename_neff_tensors_and_patch_header


def _rename_and_widen(neff_path, mapping):
    return _patch_neff(_orig_rename(neff_path, mapping))


_b2j.rename_neff_tensors_and_patch_header = _rename_and_widen


# ---------------------------------------------------------------------------
# Device program
# ---------------------------------------------------------------------------
def _build_program(kmat):
    """kmat: [NB][G] chunk counts (shared across cores)."""
    _widen_plan.clear()
    f32, bf16, i32 = mybir.dt.float32, mybir.dt.bfloat16, mybir.dt.int32
    nc = bass.Bass("TRN2", target_bir_lowering=False, debug=False,
                   num_devices=N_CORES)
    C = int(sum(kmat[b][s] for b in range(NB) for s in range(G)))
    # column offset of each (s, b) chunk group in srcw/dstw
    col0 = {}
    c = 0
    for s in range(G):
        for b in range(NB):
            col0[(s, b)] = c
            c += kmat[b][s]
    assert c == C

    xT = nc.dram_tensor("xT", [NB, IN_CH, 128], bf16, kind="ExternalInput")
    w1 = nc.dram_tensor("w1", [IN_CH, HID_CH], bf16, kind="ExternalInput")
    wmuls = nc.dram_tensor("wmuls", [HID_CH, 2 * OUT_CH], bf16, kind="ExternalInput")
    b1b = nc.dram_tensor("b1b", [128, HID_CH], f32, kind="ExternalInput")
    bmlb = nc.dram_tensor("bmlb", [128, 2 * OUT_CH], f32, kind="ExternalInput")
    dinvw = nc.dram_tensor("dinvw", [128, NB], f32, kind="ExternalInput")
    iota_in = nc.dram_tensor("iota_in", [128, 128], i32, kind="ExternalInput")
    ident_in = nc.dram_tensor("ident_in", [128, 128], bf16, kind="ExternalInput")
    srcw = nc.dram_tensor("srcw", [128, C], i32, kind="ExternalInput")
    dstw = nc.dram_tensor("dstw", [128, C], i32, kind="ExternalInput")
    out = nc.dram_tensor("out", [NL, 2 * OUT_CH], f32, kind="ExternalOutput")

    ub = [[nc.dram_tensor(f"u{p}b{s}", [SEGL, HID_CH], bf16)
           for s in range(G)] for p in (1, 2)]
    ug = [[nc.dram_tensor(f"u{p}g{s}", [SEG_ROWS, HID_CH], bf16,
                          addr_space="Shared")
           for s in range(G)] for p in (1, 2)]

    with tile.TileContext(nc) as tc:
        with (
            tc.tile_pool(name="const", bufs=1) as cp,
            tc.tile_pool(name="slab", bufs=1) as sp,
            tc.tile_pool(name="xload", bufs=4) as xp,
            tc.tile_pool(name="work", bufs=8) as wp,
            tc.tile_pool(name="mt", bufs=2) as mp,
            tc.tile_pool(name="ep", bufs=4) as ep,
            tc.tile_pool(name="psv", bufs=2, space="PSUM") as ppv,
            tc.tile_pool(name="psg", bufs=2, space="PSUM") as ppg,
            tc.tile_pool(name="pst", bufs=2, space="PSUM") as ppt,
            tc.tile_pool(name="pso", bufs=2, space="PSUM") as ppo,
        ):
            # constants
            w1_sb = [cp.tile([128, HID_CH], bf16, name=f"w1sb{k}")
                     for k in range(IN_CH // 128)]
            for k in range(IN_CH // 128):
                nc.sync.dma_start(out=w1_sb[k][:],
                                  in_=w1[k * 128:(k + 1) * 128, :])
            wml_sb = cp.tile([HID_CH, 2 * OUT_CH], bf16)
            nc.sync.dma_start(out=wml_sb[:], in_=wmuls[:])
            b1_sb = cp.tile([128, HID_CH], f32)
            nc.sync.dma_start(out=b1_sb[:], in_=b1b[:])
            bml_sb = cp.tile([128, 2 * OUT_CH], f32)
            nc.sync.dma_start(out=bml_sb[:], in_=bmlb[:])
            dinv_sb = cp.tile([128, NB], f32)
            nc.sync.dma_start(out=dinv_sb[:], in_=dinvw[:])
            iota = cp.tile([128, 128], i32)
            nc.sync.dma_start(out=iota[:], in_=iota_in[:])
            ident = cp.tile([128, 128], bf16)
            nc.sync.dma_start(out=ident[:], in_=ident_in[:])
            src_sb = sp.tile([128, C], i32)
            nc.sync.dma_start(out=src_sb[:], in_=srcw[:])
            dst_sb = sp.tile([128, C], i32)
            nc.sync.dma_start(out=dst_sb[:], in_=dstw[:])

            u1_slab = sp.tile([128, NB * HID_CH], bf16)
            u2_slab = sp.tile([128, NB * HID_CH], bf16)
            acc_slab = sp.tile([128, NB * HID_CH], f32)

            def store_rows(slab_col, b, rows, dest_pair):
                """DMA bucket b's node-major rows into segment tensors."""
                lo, hi = b * 128, b * 128 + rows
                for s in range(G):
                    s0, s1 = s * SEGL, (s + 1) * SEGL
                    a, z = max(lo, s0), min(hi, s1)
                    if a < z:
                        nc.sync.dma_start(
                            out=dest_pair[s][a - s0:z - s0, :],
                            in_=slab_col[a - lo:z - lo, :],
                        )

            def emit_ag(p, s):
                nc.gpsimd.collective_compute(
                    "AllGather", mybir.AluOpType.bypass,
                    replica_groups=[list(range(N_CORES))],
                    ins=[ub[p][s][:].opt()], outs=[ug[p][s][:].opt()],
                )

            # segment s's last producing bucket
            ag_after = [min(NB - 1, ((s + 1) * SEGL + 127) // 128 - 1)
                        for s in range(G)]

            # ---- phase 1: u1 = (x @ W1) * dinv ----
            for m in range(NB):
                rows = 128 if m < NB - 1 else LAST_ROWS
                v_ps = ppv.tile([128, HID_CH], f32, tag="v")
                for k in range(IN_CH // 128):
                    xt_t = xp.tile([128, 128], bf16, tag="xt")
                    nc.sync.dma_start(
                        out=xt_t[:], in_=xT[m, k * 128:(k + 1) * 128, :],
                    )
                    nc.tensor.matmul(
                        out=v_ps[:], lhsT=xt_t[:], rhs=w1_sb[k][:],
                        start=(k == 0), stop=(k == IN_CH // 128 - 1),
                    )
                u1_m = u1_slab[:, m * HID_CH:(m + 1) * HID_CH]
                nc.vector.tensor_scalar_mul(u1_m, v_ps[:], dinv_sb[:, m:m + 1])
                store_rows(u1_m, m, rows, ub[0])
                for s in range(G):
                    if m == ag_after[s]:
                        emit_ag(0, s)

            # ---- aggregation passes ----
            def aggregation_pass(pidx, self_slab, is_first_pass):
                for s in range(G):
                    table = ug[pidx][s]
                    for b in range(NB):
                        K = kmat[b][s]
                        if K == 0:
                            continue
                        c0 = col0[(s, b)]
                        m_slab = mp.tile([128, K * 128], bf16, tag="m")
                        nc.vector.tensor_tensor(
                            out=m_slab[:].rearrange("p (k d) -> p k d", k=K),
                            in0=dst_sb[:, c0:c0 + K].to_broadcast([128, K, 128]),
                            in1=iota[:].unsqueeze(1).broadcast_to([128, K, 128]),
                            op=mybir.AluOpType.is_equal,
                        )
                        g_ps = ppg.tile([128, HID_CH], f32, tag="g")
                        u_slab = wp.tile([128, K * HID_CH], bf16, tag="u")
                        nc.gpsimd.indirect_dma_start(
                            out=u_slab[:], out_offset=None, in_=table[:],
                            in_offset=bass.IndirectOffsetOnAxis(
                                ap=src_sb[:, c0:c0 + 1], axis=0),
                        )
                        _widen_plan.append(K)
                        for j in range(K):
                            nc.tensor.matmul(
                                out=g_ps[:],
                                lhsT=m_slab[:, j * 128:(j + 1) * 128],
                                rhs=u_slab[:, j * HID_CH:(j + 1) * HID_CH],
                                start=(j == 0), stop=(j == K - 1),
                            )
                        acc_b = acc_slab[:, b * HID_CH:(b + 1) * HID_CH]
                        if s == 0:
                            nc.vector.tensor_copy(out=acc_b, in_=g_ps[:])
                        else:
                            nc.vector.tensor_add(out=acc_b, in0=acc_b, in1=g_ps[:])
                        if s == G - 1:
                            epilogue(b, is_first_pass, self_slab)

            def epilogue(b, is_first_pass, self_slab):
                rows = 128 if b < NB - 1 else LAST_ROWS
                acc_b = acc_slab[:, b * HID_CH:(b + 1) * HID_CH]
                u_self = self_slab[:, b * HID_CH:(b + 1) * HID_CH]
                s_t = ep.tile([128, HID_CH], f32, tag="s")
                nc.vector.tensor_add(out=s_t[:], in0=acc_b, in1=u_self)
                nc.vector.tensor_scalar_mul(s_t[:], s_t[:], dinv_sb[:, b:b + 1])
                if is_first_pass:
                    nc.vector.tensor_add(out=s_t[:], in0=s_t[:], in1=b1_sb[:])
                    nc.scalar.activation(
                        s_t[:], s_t[:], mybir.ActivationFunctionType.Relu)
                    u2_m = u2_slab[:, b * HID_CH:(b + 1) * HID_CH]
                    nc.vector.tensor_scalar_mul(u2_m, s_t[:], dinv_sb[:, b:b + 1])
                    store_rows(u2_m, b, rows, ub[1])
                    for s in range(G):
                        if b == ag_after[s]:
                            emit_ag(1, s)
                else:
                    t_bf = ep.tile([128, HID_CH], bf16, tag="tb")
                    nc.vector.tensor_copy(out=t_bf[:], in_=s_t[:])
                    tT_ps = ppt.tile([HID_CH, 128], bf16, tag="tT")
                    nc.tensor.transpose(out=tT_ps[:], in_=t_bf[:], identity=ident[:])
                    tT_sb = ep.tile([HID_CH, 128], bf16, tag="tTs")
                    nc.scalar.copy(out=tT_sb[:], in_=tT_ps[:])
                    o_ps = ppo.tile([128, 2 * OUT_CH], f32, tag="o")
                    nc.tensor.matmul(
                        out=o_ps[:], lhsT=tT_sb[:], rhs=wml_sb[:],
                        start=True, stop=True,
                    )
                    o_sb = ep.tile([128, 2 * OUT_CH], f32, tag="os")
                    nc.vector.tensor_add(out=o_sb[:], in0=o_ps[:], in1=bml_sb[:])
                    nc.sync.dma_start(
                        out=out[b * 128:b * 128 + rows, :], in_=o_sb[:rows, :])

            aggregation_pass(0, u1_slab, True)
            aggregation_pass(1, u2_slab, False)

    _fix_incswdge(nc)
    _split_multiwaits(nc)
    return nc


# ---------------------------------------------------------------------------
# Host-side sharding + launch
# ---------------------------------------------------------------------------
_cache = {}


def _prep(x, edge_index, W1, b1, W_mu, b_mu, W_ls, b_ls):
    import ml_dtypes

    bf16 = ml_dtypes.bfloat16
    x = np.asarray(x, np.float32)
    src = np.asarray(edge_index[0]).astype(np.int64)
    dst = np.asarray(edge_index[1]).astype(np.int64)

    deg = np.bincount(dst, minlength=N_NODES).astype(np.float32) + 1.0
    dinv = (1.0 / np.sqrt(deg)).astype(np.float32)

    core = dst // NL
    dst_rel = dst - core * NL
    bucket = dst_rel >> 7
    dst128 = (dst_rel & 127).astype(np.int32)

    sl = src % NL
    seg = sl // SEGL
    srow = ((src // NL) * SEGL + (sl - seg * SEGL)).astype(np.int32)

    # group edges by (core, seg, bucket)
    gkey = (core * G + seg) * NB + bucket
    counts = np.bincount(gkey, minlength=N_CORES * G * NB).reshape(
        N_CORES, G, NB)
    kmat = np.maximum((counts.max(axis=0) + 127) // 128, 1).T  # [NB][G]
    kmat = kmat.astype(np.int64)

    order = np.argsort(gkey, kind="stable")
    gk_s = gkey[order]
    starts = np.zeros(N_CORES * G * NB + 1, np.int64)
    np.cumsum(counts.reshape(-1), out=starts[1:])
    rank = np.arange(len(order), dtype=np.int64) - starts[gk_s]

    # column offsets per (seg, bucket), shared by all cores
    kcum = np.zeros(G * NB + 1, np.int64)
    ks = np.stack([kmat[:, s] for s in range(G)]).reshape(-1)  # s-major
    np.cumsum(ks, out=kcum[1:])
    C = int(kcum[-1])

    cc = gk_s // (G * NB)
    sb = gk_s % (G * NB)  # seg*NB + bucket
    col = kcum[sb] + rank // 128
    row = rank % 128

    src_w = np.zeros((N_CORES, 128, C), np.int32)
    dst_w = np.full((N_CORES, 128, C), -1, np.int32)
    src_w[cc, row, col] = srow[order]
    dst_w[cc, row, col] = dst128[order]

    # Remap each (segment, bucket) group's index columns for the widened
    # gather: the patched instruction writes stream block i = p*K + j to
    # SBUF (partition p, block j), and reads index i from snake position
    # (i % 128, c0 + i // 128).
    for sb in range(G * NB):
        c0, c1 = int(kcum[sb]), int(kcum[sb + 1])
        K = c1 - c0
        if K == 0:
            continue
        blk = src_w[:, :, c0:c1]                      # [cores, 128, K]
        flat = blk.reshape(N_CORES, 128 * K)          # i = p*K + j
        src_w[:, :, c0:c1] = flat.reshape(
            N_CORES, K, 128).transpose(0, 2, 1)       # [:, i%128, i//128]

    # x^T in tile-major layout (bf16)
    xs = x.reshape(N_CORES, NL, IN_CH)
    xpad = np.zeros((N_CORES, NB * 128, IN_CH), np.float32)
    xpad[:, :NL] = xs
    xT = np.ascontiguousarray(
        np.transpose(xpad.reshape(N_CORES, NB, 128, IN_CH), (0, 1, 3, 2))
    ).astype(bf16)

    dinvw = np.ones((N_CORES, 128, NB), np.float32)
    dv = dinv.reshape(N_CORES, NL)
    for b in range(NB):
        rows = 128 if b < NB - 1 else LAST_ROWS
        dinvw[:, :rows, b] = dv[:, b * 128:b * 128 + rows]

    wmuls = np.concatenate([np.asarray(W_mu, np.float32),
                            np.asarray(W_ls, np.float32)], axis=1).astype(bf16)
    bml = np.concatenate([np.asarray(b_mu, np.float32),
                          np.asarray(b_ls, np.float32)])[None, :]
    in_map_common = {
        "w1": np.asarray(W1, np.float32).astype(bf16),
        "wmuls": wmuls,
        "b1b": np.broadcast_to(np.asarray(b1, np.float32)[None, :],
                               (128, HID_CH)).copy(),
        "bmlb": np.broadcast_to(bml, (128, 2 * OUT_CH)).astype(np.float32).copy(),
        "iota_in": np.broadcast_to(np.arange(128, dtype=np.int32)[None, :],
                                   (128, 128)).copy(),
        "ident_in": np.eye(128, dtype=np.float32).astype(bf16),
    }
    in_maps = []
    for c in range(N_CORES):
        m = dict(in_map_common)
        m["xT"] = xT[c]
        m["srcw"] = src_w[c]
        m["dstw"] = dst_w[c]
        m["dinvw"] = dinvw[c]
        in_maps.append(m)
    return kmat, in_maps


def kernel(x, edge_index, W1, b1, W_mu, b_mu, W_ls, b_ls):
    from concourse.bass_utils import run_bass_kernel_spmd

    kmat, in_maps = _prep(x, edge_index, W1, b1, W_mu, b_mu, W_ls, b_ls)
    key = kmat.tobytes()
    if key not in _cache:
        _cache[key] = _build_program(kmat)
    nc = _cache[key]
    res = run_bass_kernel_spmd(nc, in_maps, list(range(N_CORES)))
    full = np.concatenate([res.results[c]["out"] for c in range(N_CORES)], axis=0)
    return full[:, :OUT_CH].copy(), full[:, OUT_CH:].copy()
